# revision 78
# baseline (speedup 1.0000x reference)
"""Trainium2 Bass kernel for a 3-layer GAT (nn_AzureMLGraphAttentionNetwork).

Distribution strategy (8 NeuronCores, SPMD single program + per-core data):
  - Destination nodes are sharded 1250/core. Layer 1's dense transform is
    fully replicated (x is replicated anyway): every core computes the full
    record table locally, so layer 1 needs NO collective.
  - Layers 2/3: each core computes the dense transform for ITS node slice;
    the record tables are AllGathered in 2 chunks, issued from inside the
    previous layer's edge loop so the collective overlaps edge processing.
  - Each core processes only edges whose destination lands in its slice:
    edges are host-sorted by dst, grouped into 128-dst blocks, per-edge
    source records fetched with dma_gather (SWDGE descriptor gather).
  - Records are FEATURE-MAJOR ([f, h] instead of [h, f]) so the per-edge
    attention weighting is a stride-1 f16 multiply (2x DVE mode).
  - The one-hot scatter matrices are built on the (otherwise idle) GPSIMD
    engine via iota==dst compare.
  - Segment softmax is restructured: normalization after aggregation:
        out[d] = (sum_e ex_e * h[src_e]) / (sum_e ex_e)
    For layer 3 (1 head) the denominator rides as a ones-column inside the
    same accumulation matmul.
  - Attention logit pieces es/ed are folded into the dense matmul via
    host-precomputed W @ a products. ed is gathered from the LOCAL slice so
    it does not wait on the collective.

The program is identical on all cores; all per-core differences (node
slice, edge indices, scatter structure) enter as input tensors.
"""
import contextlib
import os
import sys

sys.path.insert(0, "/opt/trn_rl_repo")

import numpy as np

import concourse.bass as bass
import concourse.bacc as bacc
import concourse.mybir as mybir
import concourse.tile as tile
from concourse import library_config
from concourse.bass_utils import run_bass_kernel_spmd

F32 = mybir.dt.float32
F16 = mybir.dt.float16
I16 = mybir.dt.int16

NEG_SLOPE = 0.2
DEN_EPS = 1e-9


# --------------------------------------------------------------------------
# Configuration
# --------------------------------------------------------------------------
def full_cfg():
    return dict(
        N=10000,          # total nodes
        CORES=8,
        NLOC=1250,        # nodes per core
        HEADS=8, F=64,    # layers 1-2 heads
        IN=256, HID=512, OUT=32,
        T_BLK=36,         # edge tiles (128 edges) per 128-dst block
        CHUNK=12,         # tiles per dma_gather chunk (must divide T_BLK)
        XGRP=10,          # layer-1 dense m-tiles per x-column-group load
    )


def derived(cfg):
    d = dict(cfg)
    d["MT"] = (cfg["NLOC"] + 127) // 128          # m-tiles per core
    d["NPAD"] = d["MT"] * 128
    d["LASTM"] = cfg["NLOC"] - (d["MT"] - 1) * 128  # rows in last m-tile
    d["B"] = d["MT"]                               # dst blocks per core
    d["CPB"] = cfg["T_BLK"] // cfg["CHUNK"]        # chunks per block
    assert cfg["T_BLK"] % cfg["CHUNK"] == 0
    d["NT"] = d["B"] * cfg["T_BLK"]                # edge tiles per core
    d["NCH"] = d["NT"] // cfg["CHUNK"]             # chunks per core
    d["EPC"] = d["NT"] * 128                       # padded edges per core
    d["IDXC"] = d["EPC"] // 16
    d["IPC"] = cfg["CHUNK"] * 128 // 16            # idx cols per chunk
    d["MTF"] = (cfg["N"] + 127) // 128             # full-table m-tiles (L1)
    d["LASTF"] = cfg["N"] - (d["MTF"] - 1) * 128

    H, HID, OUT = cfg["HEADS"], cfg["HID"], cfg["OUT"]
    unit = 128  # 256B in f16 elems

    def rec_round(x):
        return ((x + unit - 1) // unit) * unit

    # layer descriptors: K=input dim, D=output dim, H=heads
    # record layout: [ed(H) | es(H) | (ones if H==1) | h(D) | pad]
    d["L"] = [
        dict(K=cfg["IN"], D=HID, H=H),
        dict(K=HID, D=HID, H=H),
        dict(K=HID, D=OUT, H=1),
    ]
    for L in d["L"]:
        L["KT"] = L["K"] // 128
        L["ONE"] = 1 if L["H"] == 1 else 0         # ones col (den fold)
        L["HOFF"] = 2 * L["H"] + L["ONE"]          # h offset in record
        L["REC"] = rec_round(L["HOFF"] + L["D"])
        L["EDE"] = unit                            # ed-gather elem (256B)
    return d


def perm_fh(H, F):
    """Column permutation mapping [h, f] order to [f, h] order.
    new_col[f*H + h] = old_col[h*F + f]."""
    idx = np.arange(H * F).reshape(H, F).T.reshape(-1)  # [f*H+h] -> h*F+f
    return idx


# --------------------------------------------------------------------------
# Host preprocessing
# --------------------------------------------------------------------------
# collective row-chunk bounds (local rows) and the C-block after which each
# chunk is issued; must be consistent between host remap and build_program.
# Last chunk kept small: it is issued after the final block, so its transfer
# is exposed at the layer boundary.
CC_BOUNDS = [0, 256, 640, 1024, 1250]
CC_BLOCKS = [1, 4, 7, 9]


def table_row_map(cfg):
    """Global node id -> chunk-major row in the gathered tables (L2/3)."""
    N, CORES, NLOC = cfg["N"], cfg["CORES"], cfg["NLOC"]
    s = np.arange(N, dtype=np.int64)
    c, r = s // NLOC, s % NLOC
    row = np.zeros(N, np.int64)
    ccb = cfg.get("CCB", CC_BOUNDS)
    for k in range(len(ccb) - 1):
        b0, b1 = ccb[k], ccb[k + 1]
        sel = (r >= b0) & (r < b1)
        row[sel] = CORES * b0 + c[sel] * (b1 - b0) + (r[sel] - b0)
    return row


def prep_edges(edge_index, cfg):
    """Per-core edge structure."""
    d = derived(cfg)
    N, CORES, NLOC = cfg["N"], cfg["CORES"], cfg["NLOC"]
    T_BLK = cfg["T_BLK"]
    rowmap = table_row_map(cfg)

    loop = np.arange(N, dtype=np.int64)
    src = np.concatenate([np.asarray(edge_index[0], np.int64), loop])
    dst = np.concatenate([np.asarray(edge_index[1], np.int64), loop])

    out = []
    for c in range(CORES):
        lo, hi = c * NLOC, (c + 1) * NLOC
        m = (dst >= lo) & (dst < hi)
        s_c, d_c = src[m], dst[m] - lo
        order = np.argsort(d_c, kind="stable")
        s_c, d_c = s_c[order], d_c[order]

        e_src = np.zeros(d["EPC"], np.int64)
        e_dstg = np.zeros(d["EPC"], np.int64)  # global dst id (L1 ED gather)
        e_dstl = np.zeros(d["EPC"], np.int64)  # local dst id (L2/3 ED gather)
        pad_mask = np.ones(d["EPC"], bool)
        dl = np.full(d["EPC"], 999.0, np.float32)
        blk_of = d_c // 128
        for b in range(d["B"]):
            sel = blk_of == b
            nb = int(sel.sum())
            cap = T_BLK * 128
            assert nb <= cap, f"block overflow: core {c} blk {b}: {nb} > {cap}"
            base = b * cap
            ss, dd = s_c[sel], d_c[sel]
            # order the block's edges by gather row: monotonic DRAM
            # addresses (and adjacent duplicates) for the record gather
            o2 = np.argsort(rowmap[ss], kind="stable")
            ss, dd = ss[o2], dd[o2]
            e_src[base:base + nb] = ss
            e_dstg[base:base + nb] = dd + lo
            e_dstl[base:base + nb] = dd
            dl[base:base + nb] = (dd - b * 128).astype(np.float32)
            pad_mask[base:base + nb] = False
            # padding: dl=999 -> zero scatter row; pad gathers use idx -1
            # (skipped by the DMA) except in the first chunks, which seed the
            # recycled gather buffers with finite data via row 0

        def wrap_idx(a):
            w = np.zeros((16, d["IDXC"]), np.int16)
            w[np.arange(d["EPC"]) % 16, np.arange(d["EPC"]) // 16] = a.astype(np.int16)
            return np.tile(w, (8, 1))

        # per-chunk gather row upper bounds (exclusive): with the src-sorted
        # edge order each chunk touches a compact row prefix, so slicing the
        # gather's in_ap to [0:bound) lets early chunks start before the
        # whole table is written (dense phase / collective overlap).
        CH128_b = cfg["CHUNK"] * 128
        rows_g = rowmap[e_src].copy()
        rows_g[pad_mask] = 0
        gb = rows_g.reshape(d["NCH"], CH128_b).max(axis=1) + 1
        rows_eg = rowmap[e_dstg].copy()
        rows_eg[pad_mask] = 0
        edg = rows_eg.reshape(d["NCH"], CH128_b).max(axis=1) + 1
        rows_el = e_dstl.copy()
        rows_el[pad_mask] = 0
        edl = rows_el.reshape(d["NCH"], CH128_b).max(axis=1) + 1

        # per-chunk / per-block real-index counts (trailing pads skipped via
        # num_idxs_reg). First chunks gather full so recycled gather buffers
        # are seeded with finite data before shorter counts leave stale rows.
        CH128 = cfg["CHUNK"] * 128
        nreal = (~pad_mask).reshape(d["NCH"], CH128).sum(axis=1)
        gcnt = np.minimum((nreal + 15) // 16 * 16, CH128).astype(np.int32)
        gcnt[:8] = CH128
        ecnt_r = (~pad_mask).reshape(d["B"], T_BLK * 128).sum(axis=1)
        ecnt = np.minimum((ecnt_r + 15) // 16 * 16, T_BLK * 128).astype(np.int32)
        ecnt[:2] = T_BLK * 128
        gcntw = np.zeros((128, d["NCH"]), np.int32)
        gcntw[0, :] = gcnt
        ecntw = np.zeros((128, d["B"]), np.int32)
        ecntw[0, :] = ecnt

        dlw = np.zeros((128, d["NT"]), np.float32)
        ii = np.arange(d["EPC"])
        dlw[ii % 128, ii // 128] = dl
        out.append(dict(srcp_idx=wrap_idx(rowmap[e_src]),
                        dstp_idx=wrap_idx(rowmap[e_dstg]),
                        dstl_idx=wrap_idx(e_dstl), dlf=dlw.astype(np.float16),
                        _gb=gb, _edg=edg, _edl=edl))
    return out


def prep_weights(inputs, cfg):
    """Shared (replicated) weight inputs, prepacked feature-major, f16."""
    d = derived(cfg)
    H, F = cfg["HEADS"], cfg["F"]
    p12 = perm_fh(H, F)  # feature-major permutation for layers 1-2 outputs

    def wa(W, a_s, a_d, heads, fh):
        Wr = np.asarray(W, np.float32).reshape(W.shape[0], heads, fh)
        WAs = np.einsum("ihf,hf->ih", Wr, np.asarray(a_s, np.float32))
        WAd = np.einsum("ihf,hf->ih", Wr, np.asarray(a_d, np.float32))
        return np.concatenate([WAd, WAs], axis=1)  # record order [ed | es]

    W1 = np.asarray(inputs["W1"], np.float32)
    W2 = np.asarray(inputs["W2"], np.float32)
    W3 = np.asarray(inputs["W3"], np.float32)

    out = {}
    # layer 1: output cols permuted
    L = d["L"][0]
    out["W1p"] = W1[:, p12].reshape(L["KT"], 128, L["D"]).astype(np.float16)
    out["WA1p"] = wa(W1, inputs["a1s"], inputs["a1d"], H, F).reshape(
        L["KT"], 128, 2 * H).astype(np.float16)
    out["brep1"] = np.broadcast_to(
        np.asarray(inputs["b1"], np.float32)[p12], (128, L["D"])).copy()
    # layer 2: input rows permuted (h from layer 1 is feature-major),
    # output cols permuted
    L = d["L"][1]
    W2p = W2[p12, :][:, p12]
    out["W2p"] = W2p.reshape(L["KT"], 128, L["D"]).astype(np.float16)
    out["WA2p"] = wa(W2, inputs["a2s"], inputs["a2d"], H, F)[p12, :].reshape(
        L["KT"], 128, 2 * H).astype(np.float16)
    out["brep2"] = np.broadcast_to(
        np.asarray(inputs["b2"], np.float32)[p12], (128, L["D"])).copy()
    # layer 3: input rows permuted, output cols unpermuted, H=1
    L = d["L"][2]
    out["W3p"] = W3[p12, :].reshape(L["KT"], 128, L["D"]).astype(np.float16)
    out["WA3p"] = wa(W3, inputs["a3s"], inputs["a3d"], 1, cfg["OUT"])[p12, :].reshape(
        L["KT"], 128, 2).astype(np.float16)
    out["brep3"] = np.broadcast_to(
        np.asarray(inputs["b3"], np.float32), (128, L["D"])).copy()

    out["ident"] = np.eye(128, dtype=np.float16)
    out["iota_c"] = np.broadcast_to(
        np.arange(128, dtype=np.float16), (128, cfg["CHUNK"], 128)).copy()
    return out


def prep_x(x, cfg):
    """Full transposed input (replicated), columns in chunk-major table row
    order so layer 1's record table is written directly in that order:
    [KT1, 128, N] f16."""
    d = derived(cfg)
    inv = np.argsort(table_row_map(cfg))
    xs = np.asarray(x, np.float32).astype(np.float16)[inv]
    return np.ascontiguousarray(xs.T.reshape(d["L"][0]["KT"], 128, cfg["N"]))


# --------------------------------------------------------------------------
# Program builder
# --------------------------------------------------------------------------
def build_program(cfg, has_bias=(False, False, False), no_collective=False,
                  tiny_gather=False, copy_split=False, bounds=None,
                  stop_phase=None, local_tables=False, slim_cc=False,
                  nq1=False, skip_consts=False, l3_no_epi=False,
                  l3_gather_only=False, nq4=True, scratch=32768,
                  gprio=250, edp_bufs=3):
    d = derived(cfg)
    N, CORES = cfg["N"], cfg["CORES"]
    NLOC, MT, NPAD, LASTM = cfg["NLOC"], d["MT"], d["NPAD"], d["LASTM"]
    MTF, LASTF = d["MTF"], d["LASTF"]
    B, T_BLK, CHUNK, CPB = d["B"], cfg["T_BLK"], cfg["CHUNK"], d["CPB"]
    NCH, IPC, XGRP = d["NCH"], d["IPC"], cfg["XGRP"]
    Ls = d["L"]
    H8, FH8 = cfg["HEADS"], cfg["F"]

    nc = bacc.Bacc(num_devices=CORES,
                   num_swdge_queues=1 if nq1 else (4 if nq4 else 2),
                   dynamic_dma_scratch_size=scratch)

    # ---- external inputs
    xTf = nc.dram_tensor("xTf", [Ls[0]["KT"], 128, N], F16, kind="ExternalInput")
    Wp, WAp, brep = [], [], []
    for i, L in enumerate(Ls):
        t = str(i + 1)
        Wp.append(nc.dram_tensor(f"W{t}p", [L["KT"], 128, L["D"]], F16, kind="ExternalInput"))
        WAp.append(nc.dram_tensor(f"WA{t}p", [L["KT"], 128, 2 * L["H"]], F16, kind="ExternalInput"))
        brep.append(nc.dram_tensor(f"brep{t}", [128, L["D"]], F32, kind="ExternalInput"))
    srcp_idx = nc.dram_tensor("srcp_idx", [128, d["IDXC"]], I16, kind="ExternalInput")
    dstp_idx = nc.dram_tensor("dstp_idx", [128, d["IDXC"]], I16, kind="ExternalInput")
    dstl_idx = nc.dram_tensor("dstl_idx", [128, d["IDXC"]], I16, kind="ExternalInput")
    dlf = nc.dram_tensor("dlf", [128, d["NT"]], F16, kind="ExternalInput")
    ident = nc.dram_tensor("ident", [128, 128], F16, kind="ExternalInput")
    iota_c = nc.dram_tensor("iota_c", [128, CHUNK, 128], F16, kind="ExternalInput")
    y_out = nc.dram_tensor("y", [NLOC, cfg["OUT"]], F32, kind="ExternalOutput")
    chain_in = nc.dram_tensor("chain", [128, 64], F32, kind="ExternalInput")
    chain_out = nc.dram_tensor("chain_out", [128, 64], F32, kind="ExternalOutput")

    # ---- internal DRAM record tables
    # layer 1 table is local-complete (dense phase replicated); 2/3 gathered
    tspace = "Local" if local_tables else "Shared"
    rec_table = [
        nc.dram_tensor("rec_table0", [N, Ls[0]["REC"]], F16),
        nc.dram_tensor("rec_table1", [N, Ls[1]["REC"]], F16, addr_space=tspace),
        nc.dram_tensor("rec_table2", [N, Ls[2]["REC"]], F16, addr_space=tspace),
    ]
    rec_slice = [None,
                 nc.dram_tensor("rec_slice1", [NLOC, Ls[1]["REC"]], F16),
                 nc.dram_tensor("rec_slice2", [NLOC, Ls[2]["REC"]], F16)]

    groups = [list(range(CORES))]

    with tile.TileContext(nc) as tc:
        with (
            tc.tile_pool(name="const", bufs=1) as const,
            tc.tile_pool(name="xf", bufs=2) as xfp,
            tc.tile_pool(name="xt", bufs=1) as xtp,
            tc.tile_pool(name="work", bufs=1) as work,
            tc.tile_pool(name="gp", bufs=6 if cfg["CHUNK"] <= 12 else 4) as gp,
            tc.tile_pool(name="edp", bufs=edp_bufs) as edp,
            tc.tile_pool(name="small", bufs=6) as small,
            tc.tile_pool(name="rt", bufs=3 if cfg["CHUNK"] <= 12 else 2) as rtp,
            tc.tile_pool(name="ps", bufs=2, space="PSUM") as ps,
            tc.tile_pool(name="psd", bufs=2, space="PSUM") as psd,
            tc.tile_pool(name="psd1", bufs=1, space="PSUM") as psd1,
            tc.tile_pool(name="pst", bufs=1, space="PSUM") as pst,
        ):
            nidx_reg = nc.gpsimd.to_reg(CHUNK * 128)
            nidx_small = nc.gpsimd.to_reg(128) if tiny_gather else None

            # timing-chain passthrough (serializes back-to-back NEFF execs)
            chn = const.tile([128, 64], F32, name="chn", tag="chn")
            nc.sync.dma_start(chn[:], chain_in[:])
            nc.sync.dma_start(chain_out[:], chn[:])

            # ---- constants into SBUF
            def load_const(ap, shape, dt=F32, name="cst"):
                t = const.tile(shape, dt, name=name, tag=name)
                if not skip_consts:
                    nc.sync.dma_start(t[:], ap[:])
                else:
                    nc.vector.memset(t[:].bitcast(F32) if dt != F32 else t[:],
                                     0.0)
                return t

            srcp_t = load_const(srcp_idx, [128, d["IDXC"]], I16, name="srcp_t")
            dstp_t = load_const(dstp_idx, [128, d["IDXC"]], I16, name="dstp_t")
            dstl_t = load_const(dstl_idx, [128, d["IDXC"]], I16, name="dstl_t")
            dlf_t = load_const(dlf, [128, d["NT"]], F16, name="dlf_t")
            id_t = load_const(ident, [128, 128], F16, name="id_t")
            iota_t = load_const(iota_c, [128, CHUNK, 128], F16, name="iota_t")

            def load_kt(ap, kt, width, name):  # [kt,128,w] dram -> [128,kt,w]
                t = const.tile([128, kt, width], F16, name=name, tag=name)
                if not skip_consts:
                    nc.sync.dma_start(t[:], ap.rearrange("k p w -> p k w"))
                else:
                    nc.vector.memset(t[:], 0.0)
                return t

            W_t = [load_kt(Wp[i], Ls[i]["KT"], Ls[i]["D"], f"W_t{i}")
                   for i in range(3)]
            WA_t = [load_kt(WAp[i], Ls[i]["KT"], 2 * Ls[i]["H"], f"WA_t{i}")
                    for i in range(3)]
            b_t = [load_const(brep[i], [128, Ls[i]["D"]], name=f"b_t{i}")
                   if has_bias[i] else None for i in range(3)]

            y_sb = work.tile([128, MT, cfg["OUT"]], F32, tag="y_sb")
            if stop_phase is not None or l3_no_epi or l3_gather_only:
                nc.vector.memset(y_sb[:], 0.0)

            # ---- pre-zero pad (and ones) columns of the record tables so
            # the per-block staging never touches them (keeps Pool/DVE free).
            # Gathered-table pads arrive via the collective from slice pads.
            zpad = const.tile([128, 128], F16, name="zpad", tag="zpad")
            nc.vector.memset(zpad[:], 0.0)
            one1 = const.tile([128, 1], F16, name="one1", tag="one1")
            nc.vector.memset(one1[:], 1.0)

            def fill_cols(tensor, nrows, c0, c1, src_tile):
                if skip_consts:
                    return
                w = c1 - c0
                for r0 in range(0, nrows, 128):
                    rr = min(128, nrows - r0)
                    nc.sync.dma_start(tensor[r0:r0 + rr, c0:c1],
                                      src_tile[0:rr, 0:w])

            L0 = Ls[0]
            for lx in (1, 2):
                Lx = Ls[lx]
                if Lx["REC"] > Lx["HOFF"] + Lx["D"]:
                    fill_cols(rec_slice[lx], NLOC, Lx["HOFF"] + Lx["D"],
                              Lx["REC"], zpad)
                if Lx["ONE"]:
                    fill_cols(rec_slice[lx], NLOC, 2 * Lx["H"], Lx["HOFF"],
                              one1)

            # ============ Layer-1 dense: full table, replicated ============
            KT0, D0, REC0, HOFF0 = L0["KT"], L0["D"], L0["REC"], L0["HOFF"]
            n_grp = (MTF + XGRP - 1) // XGRP
            if stop_phase is not None and stop_phase < 1:
                n_grp = 0
            for g in range(n_grp):
                m0 = g * XGRP
                m1 = min(m0 + XGRP, MTF)
                cols0, cols1 = m0 * 128, min(m1 * 128, N)
                xg = xfp.tile([128, KT0, XGRP * 128], F16, tag="xg", name=f"xg{g}")
                nc.sync.dma_start(
                    xg[:, :, 0:cols1 - cols0],
                    xTf[:, :, cols0:cols1].rearrange("k p w -> p k w"))
                for m in range(m0, m1):
                    lo = (m - m0) * 128
                    rows = 128 if m < MTF - 1 else LASTF
                    ph = psd.tile([128, D0], F32, tag="dbig")
                    pe = psd1.tile([128, 2 * H8], F32, tag="dsm")
                    lhs = [xg[:, k, lo:lo + 128] for k in range(KT0)]
                    for k in range(KT0):
                        nc.tensor.matmul(ph[:], lhs[k], W_t[0][:, k, :],
                                         start=(k == 0), stop=(k == KT0 - 1))
                    for k in range(KT0):
                        nc.tensor.matmul(pe[:], lhs[k], WA_t[0][:, k, :],
                                         start=(k == 0), stop=(k == KT0 - 1))
                    rt = rtp.tile([128, REC0], F16, tag="rt0", name="rt0")
                    nc.scalar.copy(rt[:, 0:2 * H8], pe[:])
                    if copy_split:
                        half0 = D0 // 2
                        nc.scalar.copy(rt[:, HOFF0:HOFF0 + half0],
                                       ph[:, 0:half0])
                        nc.vector.tensor_scalar_add(
                            rt[:, HOFF0 + half0:HOFF0 + D0],
                            ph[:, half0:D0], 0.0)
                    else:
                        nc.scalar.copy(rt[:, HOFF0:HOFF0 + D0], ph[:])
                    if REC0 > HOFF0 + D0:
                        nc.gpsimd.memset(rt[:, HOFF0 + D0:REC0], 0.0)
                    nc.sync.dma_start(rec_table[0][m * 128:m * 128 + rows, :],
                                      rt[0:rows, :])

            # ============ Edge phase (3 layers) ============
            for li, L in enumerate(Ls):
                if stop_phase is not None and li >= stop_phase - 1:
                    break
                H, D, FH, KT, REC = L["H"], L["D"], L["FH"] if "FH" in L else L["D"] // L["H"], L["KT"], L["REC"]
                FH = D // H
                HOFF, ONE = L["HOFF"], L["ONE"]
                EDE = L["EDE"]
                is_last = li == 2
                # ed gather source: L1 -> full local table w/ global ids,
                # L2/3 -> local slice w/ local ids
                ed_src = rec_table[0] if li == 0 else rec_slice[li]
                ed_idx = dstp_t if li == 0 else dstl_t
                g_idx = srcp_t

                if not is_last:
                    # next-layer dense bits (interleaved into block loop)
                    Ln = Ls[li + 1]
                    KTn = Ln["KT"]
                    xT = [xtp.tile([128, NPAD], F16, tag=f"xt{k}",
                                   name=f"xT{li}_{k}") for k in range(KTn)]
                    h_next = work.tile([128, MT, D], F16, tag="h_next",
                                       name=f"h_next{li}")

                for blk in range(B):
                    acc = ps.tile([128, HOFF - 2 * H + D], F32, tag="big",
                                  name="acc")
                    den = None
                    if not ONE:
                        den = ps.tile([128, H], F32, tag="sm", name="den")
                    for cc in range(CPB):
                        c = blk * CPB + cc
                        G = gp.tile([128, CHUNK, REC], F16, tag="G")
                        ED = edp.tile([128, CHUNK, EDE], F16, tag="ED")
                        with tc.high_priority(offset=gprio) if gprio \
                                else contextlib.nullcontext():
                            if tiny_gather:
                                nc.gpsimd.dma_gather(
                                    out_ap=G[:, 0:1, :],
                                    in_ap=rec_table[li][:, :],
                                    idxs_ap=g_idx[:, c * IPC:c * IPC + 8],
                                    num_idxs=128,
                                    num_idxs_reg=nidx_small,
                                    elem_size=REC,
                                    single_packet=False,
                                )
                                nc.gpsimd.dma_gather(
                                    out_ap=ED[:, 0:1, :],
                                    in_ap=ed_src[:, 0:EDE],
                                    idxs_ap=ed_idx[:, c * IPC:c * IPC + 8],
                                    num_idxs=128,
                                    num_idxs_reg=nidx_small,
                                    elem_size=EDE,
                                    elem_step=REC,
                                    single_packet=False,
                                    queue_num=0 if nq1 else 1,
                                )
                            else:
                                if bounds is not None:
                                    g_hi = bounds["gb"][c]
                                    e_hi = (bounds["edg"][c] if li == 0
                                            else bounds["edl"][c])
                                else:
                                    g_hi = N
                                    e_hi = N if li == 0 else NLOC
                                # with 4 rings, alternate each stream's queue
                                # per chunk (small-desc streams can be
                                # ring-throughput-bound)
                                gq, eq = 0, (0 if nq1 else 1)
                                if nq4:
                                    gq = (c % 2) * 2       # 0 / 2
                                    eq = (c % 2) * 2 + 1   # 1 / 3
                                nc.gpsimd.dma_gather(
                                    out_ap=G[:, :, :],
                                    in_ap=rec_table[li][0:g_hi, :],
                                    idxs_ap=g_idx[:, c * IPC:(c + 1) * IPC],
                                    num_idxs=CHUNK * 128,
                                    num_idxs_reg=nidx_reg,
                                    elem_size=REC,
                                    single_packet=False,
                                    queue_num=gq,
                                )
                                nc.gpsimd.dma_gather(
                                    out_ap=ED[:, :, :],
                                    in_ap=ed_src[0:e_hi, 0:EDE],
                                    idxs_ap=ed_idx[:, c * IPC:(c + 1) * IPC],
                                    num_idxs=CHUNK * 128,
                                    num_idxs_reg=nidx_reg,
                                    elem_size=EDE,
                                    elem_step=REC,
                                    single_packet=False,
                                    queue_num=eq,
                                )
                        if is_last and l3_gather_only:
                            continue
                        # ex = exp(leaky_relu(es[src] + ed[dst]))
                        z = small.tile([128, CHUNK, H], F32, tag="z")
                        nc.vector.tensor_tensor(
                            z[:], G[:, :, H:2 * H], ED[:, :, 0:H],
                            mybir.AluOpType.add)
                        z2 = small.tile([128, CHUNK, H], F32, tag="z2")
                        nc.vector.scalar_tensor_tensor(
                            z2[:], z[:], NEG_SLOPE, z[:],
                            mybir.AluOpType.mult, mybir.AluOpType.max)
                        ex = small.tile([128, CHUNK, H], F16, tag="ex")
                        nc.scalar.activation(
                            ex[:].rearrange("p a b -> p (a b)"),
                            z2[:].rearrange("p a b -> p (a b)"),
                            mybir.ActivationFunctionType.Exp)
                        # scatter one-hots (GPSIMD engine)
                        S = small.tile([128, CHUNK, 128], F16, tag="S")
                        nc.vector.tensor_tensor(
                            S[:], iota_t[:],
                            dlf_t[:, c * CHUNK:(c + 1) * CHUNK, None]
                            .broadcast_to((128, CHUNK, 128)),
                            mybir.AluOpType.is_equal)
                        # weight features by ex (in place); feature-major so
                        # the broadcast operand is stride-1 (2x DVE mode).
                        # ONE=1 (layer 3): the ones col rides along -> den.
                        gview = G[:, :, 2 * H:HOFF + D].rearrange(
                            "p t (f h) -> p t f h", h=H) if not ONE else \
                            G[:, :, 2 * H:HOFF + D]
                        if ONE:
                            nc.vector.tensor_tensor(
                                gview, gview,
                                ex[:, :, 0:1].broadcast_to((128, CHUNK, 1 + D)),
                                mybir.AluOpType.mult)
                        else:
                            nc.vector.tensor_tensor(
                                gview, gview,
                                ex[:, :, None, :].broadcast_to((128, CHUNK, FH, H)),
                                mybir.AluOpType.mult)
                        for t in range(CHUNK):
                            first = (cc == 0 and t == 0)
                            last = (cc == CPB - 1 and t == CHUNK - 1)
                            nc.tensor.matmul(acc[:], S[:, t, :],
                                             G[:, t, 2 * H:HOFF + D],
                                             start=first, stop=last)
                            if not ONE:
                                nc.tensor.matmul(den[:], S[:, t, :],
                                                 ex[:, t, :],
                                                 start=first, stop=last)

                    # ------------- epilogue for this dst block -------------
                    if not is_last:
                        # den > 0 always (every dst has a self-loop), so no
                        # eps guard; the oview mult has an f32 PSUM operand
                        # (1x DVE mode regardless), so recip stays f32 —
                        # both save a serial op in the per-block chain.
                        recip = small.tile([128, H], F32, tag="recip")
                        nc.vector.reciprocal(recip[:], den[:])
                        oview = h_next[:, blk, :].rearrange("p (f h) -> p f h", h=H)
                        nc.vector.tensor_tensor(
                            oview, acc[:].rearrange("p (f h) -> p f h", h=H),
                            recip[:, None, :].broadcast_to((128, FH, H)),
                            mybir.AluOpType.mult)
                        if has_bias[li]:
                            nc.vector.tensor_tensor(
                                h_next[:, blk, :], h_next[:, blk, :], b_t[li][:],
                                mybir.AluOpType.add)
                        nc.scalar.activation(h_next[:, blk, :], h_next[:, blk, :],
                                             mybir.ActivationFunctionType.Relu)
                        # ---- interleaved next-layer dense for m-tile blk ----
                        rows = 128 if blk < MT - 1 else LASTM
                        for k in range(KTn):
                            tp = pst.tile([128, 128], F16, tag="tp")
                            nc.tensor.transpose(
                                tp[:], h_next[:, blk, k * 128:(k + 1) * 128],
                                id_t[:])
                            nc.scalar.copy(
                                xT[k][:, blk * 128:(blk + 1) * 128], tp[:])
                        Lnx = Ls[li + 1]
                        ph = psd.tile([128, Lnx["D"]], F32, tag="dbig")
                        pe = psd1.tile([128, 2 * Lnx["H"]], F32, tag="dsm")
                        lhs = [xT[k][:, blk * 128:(blk + 1) * 128]
                               for k in range(KTn)]
                        for k in range(KTn):
                            nc.tensor.matmul(ph[:], lhs[k], W_t[li + 1][:, k, :],
                                             start=(k == 0), stop=(k == KTn - 1))
                        for k in range(KTn):
                            nc.tensor.matmul(pe[:], lhs[k], WA_t[li + 1][:, k, :],
                                             start=(k == 0), stop=(k == KTn - 1))
                        RECn, HOFFn = Lnx["REC"], Lnx["HOFF"]
                        rt = rtp.tile([128, 2 * Lnx["H"] + Lnx["D"]], F16,
                                      tag="rt1", name="rt1")
                        nc.scalar.copy(rt[:, 0:2 * Lnx["H"]], pe[:])
                        nc.scalar.copy(rt[:, 2 * Lnx["H"]:], ph[:])
                        r0 = blk * 128
                        if Lnx["ONE"]:
                            nc.sync.dma_start(
                                rec_slice[li + 1][r0:r0 + rows, 0:2 * Lnx["H"]],
                                rt[0:rows, 0:2 * Lnx["H"]])
                            nc.sync.dma_start(
                                rec_slice[li + 1][r0:r0 + rows,
                                                  HOFFn:HOFFn + Lnx["D"]],
                                rt[0:rows, 2 * Lnx["H"]:])
                        else:
                            nc.sync.dma_start(
                                rec_slice[li + 1][r0:r0 + rows,
                                                  0:HOFFn + Lnx["D"]],
                                rt[0:rows, :])
                        # ---- chunked AllGather of the next layer's table ----
                        ccblk = cfg.get("CCBLK", CC_BLOCKS)
                        ccb = cfg.get("CCB", CC_BOUNDS)
                        if blk in ccblk and not no_collective:
                            k = ccblk.index(blk)
                            b0, b1 = ccb[k], ccb[k + 1]
                            Lx1 = Ls[li + 1]
                            # ship only the used record columns; the pad
                            # columns are never read by compute
                            wcc = (Lx1["HOFF"] + Lx1["D"]) if slim_cc \
                                else Lx1["REC"]
                            nc.gpsimd.collective_compute(
                                "AllGather", mybir.AluOpType.bypass,
                                replica_groups=groups,
                                ins=[rec_slice[li + 1][b0:b1, 0:wcc]],
                                outs=[rec_table[li + 1][CORES * b0:CORES * b1,
                                                        0:wcc]],
                            )
                    elif l3_no_epi or l3_gather_only:
                        pass
                    else:
                        # layer 3: acc[:,0] = den (ones col), acc[:,1:] = h
                        recip = small.tile([128, 1], F32, tag="recip")
                        nc.vector.reciprocal(recip[:], acc[:, 0:1])
                        t3 = small.tile([128, cfg["OUT"]], F32, tag="t3")
                        nc.scalar.activation(t3[:], acc[:, 1:1 + cfg["OUT"]],
                                             mybir.ActivationFunctionType.Identity,
                                             scale=recip[:, 0:1])
                        if has_bias[li]:
                            nc.vector.tensor_tensor(t3[:], t3[:], b_t[li][:],
                                                    mybir.AluOpType.add)
                        # log_softmax without the max-subtraction: t3 is the
                        # den-normalized (softmax-averaged) GAT output, |t3|
                        # is O(1) here, so exp cannot overflow in f32 and
                        # log-softmax is shift-invariant anyway. Saves two
                        # serial DVE ops per block in the epilogue chain.
                        esc = small.tile([128, cfg["OUT"]], F32, tag="esc")
                        sm = small.tile([128, 1], F32, tag="smx")
                        nc.scalar.activation(esc[:], t3[:],
                                             mybir.ActivationFunctionType.Exp,
                                             accum_out=sm[:])
                        lnv = small.tile([128, 1], F32, tag="lnv")
                        nc.scalar.activation(lnv[:], sm[:],
                                             mybir.ActivationFunctionType.Ln)
                        nc.vector.tensor_scalar(y_sb[:, blk, :], t3[:],
                                                lnv[:, 0:1], 0.0,
                                                mybir.AluOpType.subtract,
                                                mybir.AluOpType.add)

            # ---- output
            full = MT - 1
            if full:
                nc.sync.dma_start(
                    y_out[0:full * 128, :].rearrange("(m p) c -> p m c", p=128),
                    y_sb[:, 0:full, :])
            nc.sync.dma_start(y_out[full * 128:NLOC, :], y_sb[0:LASTM, full, :])

    nc.compile()
    return nc


# --------------------------------------------------------------------------
# In-map assembly + entry point
# --------------------------------------------------------------------------
LAST_BOUNDS = None


def build_in_maps(inputs, cfg):
    global LAST_BOUNDS
    shared = prep_weights(inputs, cfg)
    shared["xTf"] = prep_x(inputs["x"], cfg)
    percore = prep_edges(inputs["edge_index"], cfg)
    in_maps = []
    gb = edg = edl = None
    for c in range(cfg["CORES"]):
        m = dict(shared)
        m.update(percore[c])
        # pool the per-chunk gather row bounds across cores (SPMD program
        # is shared, so the static in_ap slice must cover every core)
        gb = m.pop("_gb") if gb is None else np.maximum(gb, m.pop("_gb"))
        edg = m.pop("_edg") if edg is None else np.maximum(edg, m.pop("_edg"))
        edl = m.pop("_edl") if edl is None else np.maximum(edl, m.pop("_edl"))
        in_maps.append(m)
    LAST_BOUNDS = dict(gb=[int(x) for x in gb], edg=[int(x) for x in edg],
                       edl=[int(x) for x in edl])
    return in_maps


_PROGRAM_CACHE = {}
LAST_EXEC_NS = None


def kernel(**inputs):
    global LAST_EXEC_NS
    cfg = full_cfg()
    has_bias = tuple(bool(np.any(np.asarray(inputs[f"b{i}"]))) for i in (1, 2, 3))
    in_maps = build_in_maps(inputs, cfg)
    key = ("full", has_bias)
    if key not in _PROGRAM_CACHE:
        _PROGRAM_CACHE[key] = build_program(cfg, has_bias)
    nc = _PROGRAM_CACHE[key]
    for m in in_maps:
        m["chain"] = np.zeros((128, 64), np.float32)
    res = run_bass_kernel_spmd(nc, in_maps, core_ids=list(range(cfg["CORES"])))
    LAST_EXEC_NS = res.exec_time_ns
    y = np.concatenate([res.results[c]["y"] for c in range(cfg["CORES"])], axis=0)
    return y.astype(np.float32)


def _pjrt_chain_fn(nc, in_maps, n_cores):
    """Single-exec jit fn + indices of the chain input/output, for
    dispatch-level chaining (async calls serialized by the chain array)."""
    import jax
    from jax.sharding import Mesh, PartitionSpec, NamedSharding
    from jax.experimental.shard_map import shard_map
    from concourse import bass2jax

    bass2jax.install_neuronx_cc_hook()
    pname = nc.partition_id_tensor.name if nc.partition_id_tensor else None
    in_names, out_names, out_avals, zero_outs = [], [], [], []
    for alloc in nc.m.functions[0].allocations:
        if not isinstance(alloc, mybir.MemoryLocationSet):
            continue
        name = alloc.memorylocations[0].name
        if alloc.kind == "ExternalInput":
            if name != pname:
                in_names.append(name)
        elif alloc.kind == "ExternalOutput":
            shape = tuple(alloc.tensor_shape)
            dtype = mybir.dt.np(alloc.dtype)
            out_names.append(name)
            out_avals.append(jax.core.ShapedArray(shape, dtype))
            zero_outs.append(np.zeros(shape, dtype))
    all_in = list(in_names) + out_names + ([pname] if pname else [])
    ci = in_names.index("chain")
    co = out_names.index("chain_out")

    def _body(*args):
        operands = list(args)
        if pname is not None:
            operands = operands + [bass2jax.partition_id_tensor()]
        return tuple(bass2jax._bass_exec_p.bind(
            *operands, out_avals=tuple(out_avals), in_names=tuple(all_in),
            out_names=tuple(out_names), lowering_input_output_aliases=(),
            sim_require_finite=True, sim_require_nnan=True, nc=nc))

    devices = jax.devices()[:n_cores]
    mesh = Mesh(np.asarray(devices), ("core",))
    nin = len(in_names) + len(zero_outs)
    f1 = jax.jit(shard_map(_body, mesh=mesh,
                           in_specs=(PartitionSpec("core"),) * nin,
                           out_specs=(PartitionSpec("core"),) * len(out_names),
                           check_rep=False), keep_unused=True)
    concat_in = [np.concatenate([np.asarray(in_maps[c][kk])
                                 for c in range(n_cores)], axis=0)
                 for kk in in_names]
    concat_zero = [np.zeros((n_cores * z.shape[0], *z.shape[1:]), z.dtype)
                   for z in zero_outs]
    sh = NamedSharding(mesh, PartitionSpec("core"))
    dev_in = [jax.device_put(a, sh) for a in concat_in + concat_zero]
    jax.block_until_ready(dev_in)
    return f1, dev_in, ci, co


def time_kernel(inputs, iters=7, k_long=65):
    """On-device exec time via dispatch-level chaining: k async dispatches
    serialized through the chain array (device-resident), blocked once at
    the end. exec = median((T_k - T_1) / (k - 1)); dispatch noise is
    divided by k-1."""
    import time
    import jax

    cfg = full_cfg()
    has_bias = tuple(bool(np.any(np.asarray(inputs[f"b{i}"]))) for i in (1, 2, 3))
    in_maps = build_in_maps(inputs, cfg)
    key = ("full", has_bias)
    if key not in _PROGRAM_CACHE:
        _PROGRAM_CACHE[key] = build_program(cfg, has_bias)
    nc = _PROGRAM_CACHE[key]
    for m in in_maps:
        m["chain"] = np.zeros((128, 64), np.float32)
    n_cores = cfg["CORES"]
    f1, dev_in, ci, co = _pjrt_chain_fn(nc, in_maps, n_cores)

    def run_chain(k):
        args = list(dev_in)
        t0 = time.perf_counter()
        outs = None
        for _ in range(k):
            outs = f1(*args)
            args[ci] = outs[co]
        jax.block_until_ready(outs)
        return time.perf_counter() - t0

    for _ in range(2):
        run_chain(1)
        run_chain(3)
    t1s, tks, ests = [], [], []
    for _ in range(iters):
        t1 = run_chain(1)
        tk = run_chain(k_long)
        t1s.append(t1)
        tks.append(tk)
        ests.append((tk - t1) / (k_long - 1))
    med = sorted(ests)[len(ests) // 2]
    return dict(real_ms=[round(t * 1e3, 2) for t in tks],
                ctl_ms=[round(t * 1e3, 2) for t in t1s],
                est_list_us=[round(e * 1e6, 1) for e in ests],
                est_exec_s=max(med, 0.0))


if __name__ == "__main__":
    nc = build_program(full_cfg())
    print("program built ok")

